# revision 1
# baseline (speedup 1.0000x reference)
"""RGCN graph-scoring kernel for Trainium2 (8 NeuronCores, one graph per core).

Math (per graph):
  out = relu(x @ root + bias + sum_r mean_r @ W_r);  scores = out @ lin + linb
  mean_r[n] = mean of x[src_e] over edges e with dst_e == n, type_e == r.

Device strategy per core:
  1. xw[src*8 + r_local] = (x @ W_r)[src]  computed on PE into DRAM scratch
     (two halves r<8 / r>=8 so gather indices fit in int16).
  2. dma_gather edge rows z_e = xw[src_e, type_e]  (pre-transformed features).
  3. For each dst node-tile t (128 nodes): PSUM accumulator out2T[c', m]
     seeded by the root-weight matmul, then one matmul per 128-edge chunk:
       out2T += z_chunk^T @ OHa,   OHa[e, m] = alpha_e * (dstloc_e == m)
     built in one fused DVE tensor_scalar (is_equal then mult).
     alpha_e = 1/cnt(type_e, dst_e) folds the mean normalization per edge.
  4. relu+bias on ACT, then scores tile via an M=1 matmul with lin weights.

Host side only shards, sorts edges by (dst_tile, r_half) into fixed-capacity
bins (padded; pad edges have alpha=0 so they contribute nothing), and packs
index/scalar arrays in the layouts the device expects.
"""

import sys

for _p in ("/opt/trn_rl_repo", "/root/.axon_site/_ro/trn_rl_repo"):
    if _p not in sys.path:
        sys.path.insert(0, _p)

import numpy as np

import concourse.bacc as bacc
import concourse.mybir as mybir
from concourse.tile import TileContext
from concourse.bass_utils import run_bass_kernel_spmd

P = 128
B, N, C, R, E = 8, 4096, 128, 16, 65536
NT = N // P  # 32 node tiles
NH = 2  # r halves
RH = R // NH  # 8 relations per half
NBINS = NT * NH
DEF_CAP = 1280  # per-(tile, half) edge capacity; mean 1024, +8 sigma

_prog_cache = {}


def build_program(cap):
    """Build + compile the SPMD Bass program for bin capacity `cap`."""
    assert cap % P == 0
    nch = cap // P  # chunks per bin
    etot = NBINS * cap  # padded edge count
    nchunks = etot // P

    nc = bacc.Bacc("TRN2")
    f32 = mybir.dt.float32

    xT = nc.dram_tensor("xT", [P, N], f32, kind="ExternalInput")
    wcat = nc.dram_tensor("wcat", [P, R * C], f32, kind="ExternalInput")
    root = nc.dram_tensor("root", [P, C], f32, kind="ExternalInput")
    bias = nc.dram_tensor("bias", [P, 1], f32, kind="ExternalInput")
    lin = nc.dram_tensor("lin", [P, 1], f32, kind="ExternalInput")
    linb = nc.dram_tensor("linb", [1, 1], f32, kind="ExternalInput")
    iota = nc.dram_tensor("iota", [P, P], f32, kind="ExternalInput")
    gidx = nc.dram_tensor("gidx", [P, etot // 16], mybir.dt.int16, kind="ExternalInput")
    dstloc = nc.dram_tensor("dstloc", [P, nchunks], f32, kind="ExternalInput")
    alpha = nc.dram_tensor("alpha", [P, nchunks], f32, kind="ExternalInput")
    scores = nc.dram_tensor("scores", [1, N], f32, kind="ExternalOutput")

    with TileContext(nc) as tc:
        with (
            tc.tile_pool(name="const", bufs=1) as cpool,
            tc.tile_pool(name="stage", bufs=4) as spool,
            tc.tile_pool(name="z", bufs=3) as zpool,
            tc.tile_pool(name="oh", bufs=6) as ohpool,
            tc.tile_pool(name="post", bufs=4) as ppool,
            tc.tile_pool(name="pxw", bufs=1, space="PSUM") as pxw_pool,
            tc.tile_pool(name="pacc", bufs=2, space="PSUM") as pacc_pool,
            tc.tile_pool(name="plin", bufs=2, space="PSUM") as plin_pool,
            tc.tile_pool(name="dram", bufs=1, space="DRAM") as dpool,
        ):
            # ---- resident inputs ----
            xT_t = cpool.tile([P, N], f32)
            nc.sync.dma_start(out=xT_t[:], in_=xT[:])
            wcat_t = cpool.tile([P, R * C], f32)
            nc.sync.dma_start(out=wcat_t[:], in_=wcat[:])
            root_t = cpool.tile([P, C], f32)
            nc.sync.dma_start(out=root_t[:], in_=root[:])
            bias_t = cpool.tile([P, 1], f32)
            nc.sync.dma_start(out=bias_t[:], in_=bias[:])
            lin_t = cpool.tile([P, 1], f32)
            nc.sync.dma_start(out=lin_t[:], in_=lin[:])
            linb_t = cpool.tile([1, 1], f32)
            nc.sync.dma_start(out=linb_t[:], in_=linb[:])
            iota_t = cpool.tile([P, P], f32)
            nc.sync.dma_start(out=iota_t[:], in_=iota[:])
            idx_t = cpool.tile([P, etot // 16], mybir.dt.int16)
            nc.sync.dma_start(out=idx_t[:], in_=gidx[:])
            dst_t = cpool.tile([P, nchunks], f32)
            nc.sync.dma_start(out=dst_t[:], in_=dstloc[:])
            alpha_t = cpool.tile([P, nchunks], f32)
            nc.sync.dma_start(out=alpha_t[:], in_=alpha[:])

            # DRAM scratch: per-half transformed features, row = src*8 + r_local
            xw = [
                dpool.tile([N * RH, C], f32, name=f"xw{h}", tag=f"xw{h}")
                for h in range(NH)
            ]

            # ---- phase 1: xw = x @ W_r, staged out to DRAM ----
            for nchunk in range(NT):
                pxw = pxw_pool.tile([P, R * C], f32, space="PSUM")
                for g in range(4):
                    nc.tensor.matmul(
                        out=pxw[:, g * 512 : (g + 1) * 512],
                        lhsT=xT_t[:, nchunk * P : (nchunk + 1) * P],
                        rhs=wcat_t[:, g * 512 : (g + 1) * 512],
                        start=True,
                        stop=True,
                    )
                for h in range(NH):
                    stg = spool.tile([P, RH * C], f32, tag="stage")
                    nc.scalar.activation(
                        out=stg[:],
                        in_=pxw[:, h * RH * C : (h + 1) * RH * C],
                        func=mybir.ActivationFunctionType.Copy,
                    )
                    # stage [p, (rl, c')] -> xw[h] rows (nchunk*128+p)*8 + rl
                    dst_view = xw[h][:].rearrange(
                        "(nt p rl) c -> nt p rl c", nt=NT, p=P, rl=RH
                    )[nchunk]
                    nc.sync.dma_start(
                        out=dst_view,
                        in_=stg[:].rearrange("p (rl c) -> p rl c", rl=RH),
                    )

            # ---- phase 2: gather + aggregate per dst tile ----
            scores_t = cpool.tile([1, N], f32)
            for t in range(NT):
                acc = pacc_pool.tile([P, P], f32, space="PSUM", tag="acc")
                # root term seeds the accumulator (start=True clears the bank)
                nc.tensor.matmul(
                    out=acc[:],
                    lhsT=root_t[:],
                    rhs=xT_t[:, t * P : (t + 1) * P],
                    start=True,
                    stop=False,
                )
                for h in range(NH):
                    b = t * NH + h
                    z = zpool.tile([P, nch, P], f32, tag="z")
                    nc.gpsimd.dma_gather(
                        z[:],
                        xw[h][:],
                        idx_t[:, b * (cap // 16) : (b + 1) * (cap // 16)],
                        cap,
                        cap,
                        C,
                        single_packet=False,
                    )
                    for c in range(nch):
                        cidx = b * nch + c
                        oh = ohpool.tile([P, P], f32, tag="oh")
                        nc.vector.tensor_scalar(
                            out=oh[:],
                            in0=iota_t[:],
                            scalar1=dst_t[:, cidx : cidx + 1],
                            scalar2=alpha_t[:, cidx : cidx + 1],
                            op0=mybir.AluOpType.is_equal,
                            op1=mybir.AluOpType.mult,
                        )
                        nc.tensor.matmul(
                            out=acc[:],
                            lhsT=z[:, c, :],
                            rhs=oh[:],
                            start=False,
                            stop=(h == NH - 1 and c == nch - 1),
                        )
                # relu(acc + bias) -> SBUF
                relu_t = ppool.tile([P, P], f32, tag="relu")
                nc.scalar.activation(
                    out=relu_t[:],
                    in_=acc[:],
                    func=mybir.ActivationFunctionType.Relu,
                    bias=bias_t[:, :1],
                )
                plin = plin_pool.tile([P, P], f32, space="PSUM", tag="plin")
                nc.tensor.matmul(
                    out=plin[:1, :],
                    lhsT=lin_t[:],
                    rhs=relu_t[:],
                    start=True,
                    stop=True,
                )
                nc.vector.tensor_scalar(
                    out=scores_t[:1, t * P : (t + 1) * P],
                    in0=plin[:1, :],
                    scalar1=linb_t[:1, :1],
                    scalar2=None,
                    op0=mybir.AluOpType.add,
                )
            nc.sync.dma_start(out=scores[:], in_=scores_t[:])

    nc.compile()
    return nc


def _pack_core_inputs(x, ei, et, rel_w, root_w, rgcn_b, lin_w, lin_b, cap):
    """Host-side prep for one graph: sort/pad edges, pack device layouts."""
    src = ei[0].astype(np.int64)
    dst = ei[1].astype(np.int64)
    et = et.astype(np.int64)

    cnt = np.bincount(et * N + dst, minlength=R * N).astype(np.float32)
    alpha_e = 1.0 / cnt[et * N + dst]  # every edge's (r, dst) has cnt >= 1

    t_e = dst >> 7
    h_e = et >> 3
    rl_e = et & 7
    binid = t_e * NH + h_e
    order = np.argsort(binid, kind="stable")

    etot = NBINS * cap
    g = np.zeros(etot, np.int16)
    dl = np.full(etot, 999.0, np.float32)
    al = np.zeros(etot, np.float32)

    counts = np.bincount(binid, minlength=NBINS)
    if counts.max() > cap:
        raise OverflowError(int(counts.max()))
    starts = np.zeros(NBINS, np.int64)
    starts[1:] = np.cumsum(counts)[:-1]
    # position of each (sorted) edge inside the padded bin layout
    pos = np.arange(E) - starts[binid[order]] + np.arange(NBINS)[binid[order]] * cap
    g[pos] = (src[order] * 8 + rl_e[order]).astype(np.int16)
    dl[pos] = (dst[order] & 127).astype(np.float32)
    al[pos] = alpha_e[order].astype(np.float32)

    gidx = np.tile(g.reshape(-1, 16).T, (8, 1)).copy()  # [128, etot/16]
    dstloc = dl.reshape(-1, P).T.copy()  # [128, nchunks]
    alpha = al.reshape(-1, P).T.copy()

    return {
        "xT": np.ascontiguousarray(x.T),
        "wcat": np.ascontiguousarray(rel_w.transpose(1, 0, 2).reshape(C, R * C)),
        "root": np.ascontiguousarray(root_w),
        "bias": np.ascontiguousarray(rgcn_b.reshape(C, 1)),
        "lin": np.ascontiguousarray(lin_w.reshape(C, 1)),
        "linb": np.ascontiguousarray(lin_b.reshape(1, 1)),
        "iota": np.broadcast_to(np.arange(P, dtype=np.float32), (P, P)).copy(),
        "gidx": gidx,
        "dstloc": dstloc,
        "alpha": alpha,
    }


def kernel(node_features, edge_index, edge_type, rel_weight, root_weight,
           rgcn_bias, lin_weight, lin_bias, **_ignored):
    node_features = np.asarray(node_features, np.float32)
    edge_index = np.asarray(edge_index)
    edge_type = np.asarray(edge_type)
    rel_weight = np.asarray(rel_weight, np.float32)
    root_weight = np.asarray(root_weight, np.float32)
    rgcn_bias = np.asarray(rgcn_bias, np.float32)
    lin_weight = np.asarray(lin_weight, np.float32)
    lin_bias = np.asarray(lin_bias, np.float32)

    cap = DEF_CAP
    while True:
        try:
            in_maps = [
                _pack_core_inputs(
                    node_features[b], edge_index[b], edge_type[b], rel_weight,
                    root_weight, rgcn_bias, lin_weight, lin_bias, cap,
                )
                for b in range(B)
            ]
            break
        except OverflowError as e:
            cap = ((int(e.args[0]) + P - 1) // P + 1) * P

    if cap not in _prog_cache:
        _prog_cache[cap] = build_program(cap)
    nc = _prog_cache[cap]

    res = run_bass_kernel_spmd(nc, in_maps, core_ids=list(range(B)))
    out = np.stack([res.results[b]["scores"].reshape(N) for b in range(B)])
    return out.astype(np.float32)


def kernel_profiled(node_features, edge_index, edge_type, rel_weight,
                    root_weight, rgcn_bias, lin_weight, lin_bias, **_ignored):
    """Run once with NTFF tracing; returns exec_time_ns (or None)."""
    import tempfile

    in_maps = [
        _pack_core_inputs(
            np.asarray(node_features, np.float32)[b], np.asarray(edge_index)[b],
            np.asarray(edge_type)[b], np.asarray(rel_weight, np.float32),
            np.asarray(root_weight, np.float32), np.asarray(rgcn_bias, np.float32),
            np.asarray(lin_weight, np.float32), np.asarray(lin_bias, np.float32),
            DEF_CAP,
        )
        for b in range(B)
    ]
    if DEF_CAP not in _prog_cache:
        _prog_cache[DEF_CAP] = build_program(DEF_CAP)
    nc = _prog_cache[DEF_CAP]
    tmpdir = tempfile.mkdtemp(prefix="rgcn_prof_")
    res = run_bass_kernel_spmd(
        nc, in_maps, core_ids=list(range(B)), trace=True, tmpdir=tmpdir
    )
    print(f"profile artifacts in {tmpdir}")
    return res.exec_time_ns



# revision 3
# speedup vs baseline: 2.9539x; 2.9539x over previous
"""RGCN graph-scoring kernel for Trainium2 (8 NeuronCores, one graph per core).

Math (per graph):
  out = relu(x @ root + bias + sum_r mean_r @ W_r);  scores = out @ lin + linb
  mean_r[n] = mean of x[src_e] over edges e with dst_e == n, type_e == r.

Device strategy per core (v2 — bf16 + 4 SWDGE queues + host-built one-hots):
  1. Phase 1: xw[src*8 + r_local] = (x @ W_r)[src] in bf16, staged to DRAM
     (two halves r<8 / r>=8 so gather indices fit in int16). PSUM->SBUF
     copies alternate between the Scalar and Vector engines.
  2. Phase 2, per dst tile t: dma_gather edge rows z_e = xw[src_e, type_e]
     (gathers round-robin across 4 SWDGE queues so descriptor generation
     runs on multiple Q7 core pairs concurrently), then accumulate
       acc[c', m] += z_chunk^T @ OH_chunk
     where OH_chunk[e, m] = alpha_e * (dstloc_e == m) is PRECOMPUTED ON THE
     HOST in bf16 and DMA-streamed (alpha_e = 1/cnt(type_e, dst_e) folds the
     mean normalization; padding slots have alpha 0 and index 0).
     acc is seeded by the root-weight matmul; relu+bias on ACT; scores via
     an M=1 matmul with the linear head.

Host side shards graphs across cores, sorts edges by (dst_tile, r_half) into
per-bin chunk counts shared across all 8 cores (max over cores, rounded up to
128) so the SPMD program is identical, and packs index/one-hot arrays.
"""

import sys

for _p in ("/opt/trn_rl_repo", "/root/.axon_site/_ro/trn_rl_repo"):
    if _p not in sys.path:
        sys.path.insert(0, _p)

import numpy as np
import ml_dtypes

import concourse.bacc as bacc
import concourse.mybir as mybir
from concourse.tile import TileContext
from concourse.bass_utils import run_bass_kernel_spmd

P = 128
B, N, C, R, E = 8, 4096, 128, 16, 65536
NT = N // P  # 32 dst node tiles
NH = 2  # r halves (int16 gather index limit: src*8+rl < 32768)
RH = R // NH  # 8 relations per half
NBINS = NT * NH
NQ = 4  # SWDGE queues

BF16 = ml_dtypes.bfloat16

_prog_cache = {}


def build_program(nch):
    """Build + compile the SPMD Bass program for per-bin chunk counts `nch`
    (tuple of NBINS ints, shared by all cores)."""
    nch = tuple(int(v) for v in nch)
    G = sum(nch)  # total 128-edge chunks
    off = np.zeros(NBINS + 1, np.int64)
    off[1:] = np.cumsum(nch)

    nc = bacc.Bacc("TRN2", num_swdge_queues=NQ)
    f32 = mybir.dt.float32
    bf16 = mybir.dt.bfloat16

    xT = nc.dram_tensor("xT", [P, N], bf16, kind="ExternalInput")
    wcat = nc.dram_tensor("wcat", [P, R * C], bf16, kind="ExternalInput")
    root = nc.dram_tensor("root", [P, C], bf16, kind="ExternalInput")
    bias = nc.dram_tensor("bias", [P, 1], f32, kind="ExternalInput")
    lin = nc.dram_tensor("lin", [P, 1], bf16, kind="ExternalInput")
    linb = nc.dram_tensor("linb", [1, 1], f32, kind="ExternalInput")
    gidx = nc.dram_tensor("gidx", [P, G * 8], mybir.dt.int16, kind="ExternalInput")
    oh = nc.dram_tensor("oh", [P, G * P], bf16, kind="ExternalInput")
    scores = nc.dram_tensor("scores", [1, N], f32, kind="ExternalOutput")

    with TileContext(nc) as tc:
        with (
            tc.tile_pool(name="const", bufs=1) as cpool,
            tc.tile_pool(name="stage", bufs=4) as spool,
            tc.tile_pool(name="z", bufs=8) as zpool,
            tc.tile_pool(name="oh", bufs=3) as ohpool,
            tc.tile_pool(name="post", bufs=4) as ppool,
            tc.tile_pool(name="pxw", bufs=2, space="PSUM") as pxw_pool,
            tc.tile_pool(name="pacc", bufs=2, space="PSUM") as pacc_pool,
            tc.tile_pool(name="plin", bufs=2, space="PSUM") as plin_pool,
            tc.tile_pool(name="dram", bufs=1, space="DRAM") as dpool,
        ):
            # ---- resident inputs ----
            xT_t = cpool.tile([P, N], bf16)
            nc.sync.dma_start(out=xT_t[:], in_=xT[:])
            wcat_t = cpool.tile([P, R * C], bf16)
            nc.sync.dma_start(out=wcat_t[:], in_=wcat[:])
            root_t = cpool.tile([P, C], bf16)
            nc.sync.dma_start(out=root_t[:], in_=root[:])
            bias_t = cpool.tile([P, 1], f32)
            nc.sync.dma_start(out=bias_t[:], in_=bias[:])
            lin_t = cpool.tile([P, 1], bf16)
            nc.sync.dma_start(out=lin_t[:], in_=lin[:])
            linb_t = cpool.tile([1, 1], f32)
            nc.sync.dma_start(out=linb_t[:], in_=linb[:])
            idx_t = cpool.tile([P, G * 8], mybir.dt.int16)
            nc.sync.dma_start(out=idx_t[:], in_=gidx[:])

            # DRAM scratch: per-half transformed features, row = src*8 + r_local
            xw = [
                dpool.tile([N * RH, C], bf16, name=f"xw{h}", tag=f"xw{h}")
                for h in range(NH)
            ]

            # ---- phase 1: xw = x @ W_r (bf16), staged out to DRAM ----
            for st in range(NT):
                for h in range(NH):
                    pxw = pxw_pool.tile([P, RH * C], f32, space="PSUM", tag="pxw")
                    for g in range(2):
                        nc.tensor.matmul(
                            out=pxw[:, g * 512 : (g + 1) * 512],
                            lhsT=xT_t[:, st * P : (st + 1) * P],
                            rhs=wcat_t[:, h * RH * C + g * 512 : h * RH * C + (g + 1) * 512],
                            start=True,
                            stop=True,
                        )
                    stg = spool.tile([P, RH * C], bf16, tag="stage")
                    if h == 0:
                        nc.scalar.activation(
                            out=stg[:],
                            in_=pxw[:],
                            func=mybir.ActivationFunctionType.Copy,
                        )
                    else:
                        nc.vector.tensor_scalar(
                            out=stg[:],
                            in0=pxw[:],
                            scalar1=0.0,
                            scalar2=None,
                            op0=mybir.AluOpType.add,
                        )
                    dst_view = xw[h][:].rearrange(
                        "(nt p rl) c -> nt p rl c", nt=NT, p=P, rl=RH
                    )[st]
                    nc.sync.dma_start(
                        out=dst_view,
                        in_=stg[:].rearrange("p (rl c) -> p rl c", rl=RH),
                    )

            # ---- phase 2: gather + aggregate per dst tile ----
            scores_t = cpool.tile([1, N], f32)
            for t in range(NT):
                b0, b1 = t * NH, t * NH + 1
                tch = nch[b0] + nch[b1]  # chunks for this tile
                oh_t = ohpool.tile([P, tch * P], bf16, tag="oh")
                nc.sync.dma_start(
                    out=oh_t[:], in_=oh[:, off[b0] * P : (off[b0] + tch) * P]
                )
                acc = pacc_pool.tile([P, P], f32, space="PSUM", tag="acc")
                # root term seeds the accumulator (start=True clears the bank)
                nc.tensor.matmul(
                    out=acc[:],
                    lhsT=root_t[:],
                    rhs=xT_t[:, t * P : (t + 1) * P],
                    start=True,
                    stop=False,
                )
                for h in range(NH):
                    b = t * NH + h
                    cap = nch[b] * P
                    z = zpool.tile([P, nch[b], C], bf16, tag="z")
                    nc.gpsimd.dma_gather(
                        z[:],
                        xw[h][:],
                        idx_t[:, off[b] * 8 : off[b] * 8 + cap // 16],
                        cap,
                        cap,
                        C,
                        single_packet=False,
                        queue_num=b % NQ,
                    )
                    for j in range(nch[b]):
                        g_loc = (off[b] - off[b0]) + j
                        nc.tensor.matmul(
                            out=acc[:],
                            lhsT=z[:, j, :],
                            rhs=oh_t[:, g_loc * P : (g_loc + 1) * P],
                            start=False,
                            stop=(h == NH - 1 and j == nch[b] - 1),
                        )
                # relu(acc + bias) -> SBUF bf16
                relu_t = ppool.tile([P, P], bf16, tag="relu")
                nc.scalar.activation(
                    out=relu_t[:],
                    in_=acc[:],
                    func=mybir.ActivationFunctionType.Relu,
                    bias=bias_t[:, :1],
                )
                plin = plin_pool.tile([P, P], f32, space="PSUM", tag="plin")
                nc.tensor.matmul(
                    out=plin[:1, :],
                    lhsT=lin_t[:],
                    rhs=relu_t[:],
                    start=True,
                    stop=True,
                )
                nc.vector.tensor_scalar(
                    out=scores_t[:1, t * P : (t + 1) * P],
                    in0=plin[:1, :],
                    scalar1=linb_t[:1, :1],
                    scalar2=None,
                    op0=mybir.AluOpType.add,
                )
            nc.sync.dma_start(out=scores[:], in_=scores_t[:])

    nc.compile()
    return nc


def _bin_edges(ei, et):
    """Per-graph bin ids and per-edge fields (no padding decisions here)."""
    src = ei[0].astype(np.int64)
    dst = ei[1].astype(np.int64)
    et = et.astype(np.int64)
    cnt = np.bincount(et * N + dst, minlength=R * N).astype(np.float32)
    alpha_e = 1.0 / cnt[et * N + dst]
    t_e = dst >> 7
    h_e = et >> 3
    rl_e = et & 7
    binid = t_e * NH + h_e
    return src, dst, rl_e, alpha_e, binid


def _pack_core_inputs(x, src, dst, rl_e, alpha_e, binid, nch, off,
                      rel_w, root_w, rgcn_b, lin_w, lin_b):
    """Host-side prep for one graph given shared per-bin chunk counts."""
    G = int(off[-1])
    order = np.argsort(binid, kind="stable")
    counts = np.bincount(binid, minlength=NBINS)
    starts = np.zeros(NBINS, np.int64)
    starts[1:] = np.cumsum(counts)[:-1]
    # slot of each (sorted) edge inside the padded chunk layout
    pos = np.arange(E) - starts[binid[order]] + off[binid[order]] * P

    g = np.zeros(G * P, np.int16)  # pad slots gather row 0 (alpha 0)
    g[pos] = (src[order] * 8 + rl_e[order]).astype(np.int16)
    gidx = np.tile(g.reshape(-1, 16).T, (8, 1)).copy()

    A = np.zeros((G * P, P), np.float32)
    A[pos, (dst[order] & 127)] = alpha_e[order]
    oh = np.ascontiguousarray(
        A.reshape(G, P, P).transpose(1, 0, 2).reshape(P, G * P)
    ).astype(BF16)

    return {
        "xT": np.ascontiguousarray(x.T).astype(BF16),
        "wcat": np.ascontiguousarray(
            rel_w.transpose(1, 0, 2).reshape(C, R * C)
        ).astype(BF16),
        "root": np.ascontiguousarray(root_w).astype(BF16),
        "bias": np.ascontiguousarray(rgcn_b.reshape(C, 1)).astype(np.float32),
        "lin": np.ascontiguousarray(lin_w.reshape(C, 1)).astype(BF16),
        "linb": np.ascontiguousarray(lin_b.reshape(1, 1)).astype(np.float32),
        "gidx": gidx,
        "oh": oh,
    }


def _prep(node_features, edge_index, edge_type, rel_weight, root_weight,
          rgcn_bias, lin_weight, lin_bias):
    node_features = np.asarray(node_features, np.float32)
    edge_index = np.asarray(edge_index)
    edge_type = np.asarray(edge_type)
    rel_weight = np.asarray(rel_weight, np.float32)
    root_weight = np.asarray(root_weight, np.float32)
    rgcn_bias = np.asarray(rgcn_bias, np.float32)
    lin_weight = np.asarray(lin_weight, np.float32)
    lin_bias = np.asarray(lin_bias, np.float32)

    per_core = [
        _bin_edges(edge_index[b], edge_type[b]) for b in range(B)
    ]
    counts = np.stack(
        [np.bincount(pc[4], minlength=NBINS) for pc in per_core]
    )  # [B, NBINS]
    nch = tuple(int(v) for v in np.maximum(
        1, -(-counts.max(axis=0) // P)
    ))  # shared per-bin chunk counts
    off = np.zeros(NBINS + 1, np.int64)
    off[1:] = np.cumsum(nch)

    in_maps = [
        _pack_core_inputs(
            node_features[b], *per_core[b], nch, off,
            rel_weight, root_weight, rgcn_bias, lin_weight, lin_bias,
        )
        for b in range(B)
    ]
    return nch, in_maps


def kernel(node_features, edge_index, edge_type, rel_weight, root_weight,
           rgcn_bias, lin_weight, lin_bias, **_ignored):
    nch, in_maps = _prep(node_features, edge_index, edge_type, rel_weight,
                         root_weight, rgcn_bias, lin_weight, lin_bias)
    if nch not in _prog_cache:
        _prog_cache[nch] = build_program(nch)
    nc = _prog_cache[nch]
    res = run_bass_kernel_spmd(nc, in_maps, core_ids=list(range(B)))
    out = np.stack([res.results[b]["scores"].reshape(N) for b in range(B)])
    return out.astype(np.float32)


def kernel_profiled(node_features, edge_index, edge_type, rel_weight,
                    root_weight, rgcn_bias, lin_weight, lin_bias, **_ignored):
    """Run once with NTFF tracing; returns exec_time_ns (or None)."""
    import tempfile

    nch, in_maps = _prep(node_features, edge_index, edge_type, rel_weight,
                         root_weight, rgcn_bias, lin_weight, lin_bias)
    if nch not in _prog_cache:
        _prog_cache[nch] = build_program(nch)
    nc = _prog_cache[nch]
    tmpdir = tempfile.mkdtemp(prefix="rgcn_prof_")
    res = run_bass_kernel_spmd(
        nc, in_maps, core_ids=list(range(B)), trace=True, tmpdir=tmpdir
    )
    print(f"profile artifacts in {tmpdir}")
    return res.exec_time_ns


# revision 9
# speedup vs baseline: 2.9695x; 1.0053x over previous
"""RGCN graph-scoring kernel for Trainium2 (8 NeuronCores, one graph per core).

Math (per graph):
  out = relu(x @ root + bias + sum_r mean_r @ W_r);  scores = out @ lin + linb
  mean_r[n] = mean of x[src_e] over edges e with dst_e == n, type_e == r.

Device strategy per core (v2 — bf16 + 4 SWDGE queues + host-built one-hots):
  1. Phase 1: xw[src*8 + r_local] = (x @ W_r)[src] in bf16, staged to DRAM
     (two halves r<8 / r>=8 so gather indices fit in int16). PSUM->SBUF
     copies alternate between the Scalar and Vector engines.
  2. Phase 2, per dst tile t: dma_gather edge rows z_e = xw[src_e, type_e]
     (gathers round-robin across 4 SWDGE queues so descriptor generation
     runs on multiple Q7 core pairs concurrently), then accumulate
       acc[c', m] += z_chunk^T @ OH_chunk
     where OH_chunk[e, m] = alpha_e * (dstloc_e == m) is PRECOMPUTED ON THE
     HOST in bf16 and DMA-streamed (alpha_e = 1/cnt(type_e, dst_e) folds the
     mean normalization; padding slots have alpha 0 and index 0).
     acc is seeded by the root-weight matmul; relu+bias on ACT; scores via
     an M=1 matmul with the linear head.

Host side shards graphs across cores, sorts edges by (dst_tile, r_half) into
per-bin chunk counts shared across all 8 cores (max over cores, rounded up to
128) so the SPMD program is identical, and packs index/one-hot arrays.
"""

import sys

for _p in ("/opt/trn_rl_repo", "/root/.axon_site/_ro/trn_rl_repo"):
    if _p not in sys.path:
        sys.path.insert(0, _p)

import numpy as np
import ml_dtypes

import concourse.bacc as bacc
import concourse.mybir as mybir
from concourse.tile import TileContext
from concourse.bass_utils import run_bass_kernel_spmd

P = 128
B, N, C, R, E = 8, 4096, 128, 16, 65536
NT = N // P  # 32 dst node tiles
NH = 2  # r halves (int16 gather index limit: src*8+rl < 32768)
RH = R // NH  # 8 relations per half
NBINS = NT * NH
NQ = 4  # SWDGE queues

BF16 = ml_dtypes.bfloat16

_prog_cache = {}


def build_program(nch):
    """Build + compile the SPMD Bass program for per-bin chunk counts `nch`
    (tuple of NBINS ints, shared by all cores)."""
    nch = tuple(int(v) for v in nch)
    G = sum(nch)  # total 128-edge chunks
    off = np.zeros(NBINS + 1, np.int64)
    off[1:] = np.cumsum(nch)

    nc = bacc.Bacc("TRN2", num_swdge_queues=NQ)
    f32 = mybir.dt.float32
    bf16 = mybir.dt.bfloat16

    xT = nc.dram_tensor("xT", [P, N], bf16, kind="ExternalInput")
    wcat = nc.dram_tensor("wcat", [P, R * C], bf16, kind="ExternalInput")
    root = nc.dram_tensor("root", [P, C], bf16, kind="ExternalInput")
    bias = nc.dram_tensor("bias", [P, 1], f32, kind="ExternalInput")
    lin = nc.dram_tensor("lin", [P, 1], bf16, kind="ExternalInput")
    linb = nc.dram_tensor("linb", [1, 1], f32, kind="ExternalInput")
    gidx = nc.dram_tensor("gidx", [P, G * 8], mybir.dt.int16, kind="ExternalInput")
    oh = nc.dram_tensor("oh", [P, G * P], bf16, kind="ExternalInput")
    scores = nc.dram_tensor("scores", [1, N], f32, kind="ExternalOutput")

    with TileContext(nc) as tc:
        with (
            tc.tile_pool(name="const", bufs=1) as cpool,
            tc.tile_pool(name="stage", bufs=4) as spool,
            tc.tile_pool(name="z0", bufs=NT) as zpool0,
            tc.tile_pool(name="z1", bufs=6) as zpool1,
            tc.tile_pool(name="oh", bufs=3) as ohpool,
            tc.tile_pool(name="post", bufs=4) as ppool,
            tc.tile_pool(name="pxw", bufs=2, space="PSUM") as pxw_pool,
            tc.tile_pool(name="pacc", bufs=2, space="PSUM") as pacc_pool,
            tc.tile_pool(name="plin", bufs=2, space="PSUM") as plin_pool,
            tc.tile_pool(name="dram", bufs=1, space="DRAM") as dpool,
        ):
            # ---- resident inputs ----
            xT_t = cpool.tile([P, N], bf16)
            nc.sync.dma_start(out=xT_t[:], in_=xT[:])
            wcat_t = cpool.tile([P, R * C], bf16)
            nc.sync.dma_start(out=wcat_t[:], in_=wcat[:])
            root_t = cpool.tile([P, C], bf16)
            nc.sync.dma_start(out=root_t[:], in_=root[:])
            bias_t = cpool.tile([P, 1], f32)
            nc.sync.dma_start(out=bias_t[:], in_=bias[:])
            lin_t = cpool.tile([P, 1], bf16)
            nc.sync.dma_start(out=lin_t[:], in_=lin[:])
            linb_t = cpool.tile([1, 1], f32)
            nc.sync.dma_start(out=linb_t[:], in_=linb[:])
            idx_t = cpool.tile([P, G * 8], mybir.dt.int16)
            nc.sync.dma_start(out=idx_t[:], in_=gidx[:])

            # DRAM scratch: per-half transformed features, row = src*8 + r_local
            xw = [
                dpool.tile([N * RH, C], bf16, name=f"xw{h}", tag=f"xw{h}")
                for h in range(NH)
            ]

            # ---- phase 1: xw = x @ W_r (bf16), staged out to DRAM ----
            # h-major so all of xw[0] lands first and the h=0 gathers can
            # start while the h=1 half is still being computed.
            for h in range(NH):
                for st in range(NT):
                    pxw = pxw_pool.tile([P, RH * C], f32, space="PSUM", tag="pxw")
                    for g in range(2):
                        nc.tensor.matmul(
                            out=pxw[:, g * 512 : (g + 1) * 512],
                            lhsT=xT_t[:, st * P : (st + 1) * P],
                            rhs=wcat_t[:, h * RH * C + g * 512 : h * RH * C + (g + 1) * 512],
                            start=True,
                            stop=True,
                        )
                    stg = spool.tile([P, RH * C], bf16, tag="stage")
                    # split the PSUM->SBUF cast across ACT and DVE so neither
                    # engine gates the PE
                    nc.scalar.activation(
                        out=stg[:, :512],
                        in_=pxw[:, :512],
                        func=mybir.ActivationFunctionType.Copy,
                    )
                    nc.vector.tensor_scalar(
                        out=stg[:, 512:],
                        in0=pxw[:, 512:],
                        scalar1=0.0,
                        scalar2=None,
                        op0=mybir.AluOpType.add,
                    )
                    dst_view = xw[h][:].rearrange(
                        "(nt p rl) c -> nt p rl c", nt=NT, p=P, rl=RH
                    )[st]
                    nc.sync.dma_start(
                        out=dst_view,
                        in_=stg[:].rearrange("p (rl c) -> p rl c", rl=RH),
                    )

            # ---- phase 2: gather + aggregate per dst tile ----
            # All h=0 gathers are issued first: they only depend on xw[0], so
            # Q7 descriptor generation overlaps with phase 1's h=1 half.
            z_h0 = []
            for t in range(NT):
                b = t * NH
                cap = nch[b] * P
                z = zpool0.tile([P, nch[b], C], bf16, tag="z0")
                nc.gpsimd.dma_gather(
                    z[:],
                    xw[0][:],
                    idx_t[:, off[b] * 8 : off[b] * 8 + cap // 16],
                    cap,
                    cap,
                    C,
                    single_packet=False,
                    queue_num=t % NQ,
                )
                z_h0.append(z)

            scores_t = cpool.tile([1, N], f32)
            for t in range(NT):
                b0, b1 = t * NH, t * NH + 1
                tch = nch[b0] + nch[b1]  # chunks for this tile
                oh_t = ohpool.tile([P, tch * P], bf16, tag="oh")
                nc.sync.dma_start(
                    out=oh_t[:], in_=oh[:, off[b0] * P : (off[b0] + tch) * P]
                )
                cap1 = nch[b1] * P
                z1 = zpool1.tile([P, nch[b1], C], bf16, tag="z1")
                nc.gpsimd.dma_gather(
                    z1[:],
                    xw[1][:],
                    idx_t[:, off[b1] * 8 : off[b1] * 8 + cap1 // 16],
                    cap1,
                    cap1,
                    C,
                    single_packet=False,
                    queue_num=t % NQ,
                )
                acc = pacc_pool.tile([P, P], f32, space="PSUM", tag="acc")
                # root term seeds the accumulator (start=True clears the bank)
                nc.tensor.matmul(
                    out=acc[:],
                    lhsT=root_t[:],
                    rhs=xT_t[:, t * P : (t + 1) * P],
                    start=True,
                    stop=False,
                )
                for h, zt in ((0, z_h0[t]), (1, z1)):
                    b = t * NH + h
                    for j in range(nch[b]):
                        g_loc = (off[b] - off[b0]) + j
                        nc.tensor.matmul(
                            out=acc[:],
                            lhsT=zt[:, j, :],
                            rhs=oh_t[:, g_loc * P : (g_loc + 1) * P],
                            start=False,
                            stop=(h == NH - 1 and j == nch[b] - 1),
                        )
                # relu(acc + bias) -> SBUF bf16
                relu_t = ppool.tile([P, P], bf16, tag="relu")
                nc.scalar.activation(
                    out=relu_t[:],
                    in_=acc[:],
                    func=mybir.ActivationFunctionType.Relu,
                    bias=bias_t[:, :1],
                )
                plin = plin_pool.tile([P, P], f32, space="PSUM", tag="plin")
                nc.tensor.matmul(
                    out=plin[:1, :],
                    lhsT=lin_t[:],
                    rhs=relu_t[:],
                    start=True,
                    stop=True,
                )
                nc.vector.tensor_scalar(
                    out=scores_t[:1, t * P : (t + 1) * P],
                    in0=plin[:1, :],
                    scalar1=linb_t[:1, :1],
                    scalar2=None,
                    op0=mybir.AluOpType.add,
                )
            nc.sync.dma_start(out=scores[:], in_=scores_t[:])

    nc.compile()
    return nc


def _bin_edges(ei, et):
    """Per-graph bin ids and per-edge fields (no padding decisions here)."""
    src = ei[0].astype(np.int64)
    dst = ei[1].astype(np.int64)
    et = et.astype(np.int64)
    cnt = np.bincount(et * N + dst, minlength=R * N).astype(np.float32)
    alpha_e = 1.0 / cnt[et * N + dst]
    t_e = dst >> 7
    h_e = et >> 3
    rl_e = et & 7
    binid = t_e * NH + h_e
    return src, dst, rl_e, alpha_e, binid


def _pack_core_inputs(x, src, dst, rl_e, alpha_e, binid, nch, off,
                      rel_w, root_w, rgcn_b, lin_w, lin_b):
    """Host-side prep for one graph given shared per-bin chunk counts."""
    G = int(off[-1])
    order = np.argsort(binid, kind="stable")
    counts = np.bincount(binid, minlength=NBINS)
    starts = np.zeros(NBINS, np.int64)
    starts[1:] = np.cumsum(counts)[:-1]
    # slot of each (sorted) edge inside the padded chunk layout
    pos = np.arange(E) - starts[binid[order]] + off[binid[order]] * P

    g = np.zeros(G * P, np.int16)  # pad slots gather row 0 (alpha 0)
    g[pos] = (src[order] * 8 + rl_e[order]).astype(np.int16)
    gidx = np.tile(g.reshape(-1, 16).T, (8, 1)).copy()

    A = np.zeros((G * P, P), np.float32)
    A[pos, (dst[order] & 127)] = alpha_e[order]
    oh = np.ascontiguousarray(
        A.reshape(G, P, P).transpose(1, 0, 2).reshape(P, G * P)
    ).astype(BF16)

    return {
        "xT": np.ascontiguousarray(x.T).astype(BF16),
        "wcat": np.ascontiguousarray(
            rel_w.transpose(1, 0, 2).reshape(C, R * C)
        ).astype(BF16),
        "root": np.ascontiguousarray(root_w).astype(BF16),
        "bias": np.ascontiguousarray(rgcn_b.reshape(C, 1)).astype(np.float32),
        "lin": np.ascontiguousarray(lin_w.reshape(C, 1)).astype(BF16),
        "linb": np.ascontiguousarray(lin_b.reshape(1, 1)).astype(np.float32),
        "gidx": gidx,
        "oh": oh,
    }


def _prep(node_features, edge_index, edge_type, rel_weight, root_weight,
          rgcn_bias, lin_weight, lin_bias):
    node_features = np.asarray(node_features, np.float32)
    edge_index = np.asarray(edge_index)
    edge_type = np.asarray(edge_type)
    rel_weight = np.asarray(rel_weight, np.float32)
    root_weight = np.asarray(root_weight, np.float32)
    rgcn_bias = np.asarray(rgcn_bias, np.float32)
    lin_weight = np.asarray(lin_weight, np.float32)
    lin_bias = np.asarray(lin_bias, np.float32)

    per_core = [
        _bin_edges(edge_index[b], edge_type[b]) for b in range(B)
    ]
    counts = np.stack(
        [np.bincount(pc[4], minlength=NBINS) for pc in per_core]
    )  # [B, NBINS]
    nch = tuple(int(v) for v in np.maximum(
        1, -(-counts.max(axis=0) // P)
    ))  # shared per-bin chunk counts
    off = np.zeros(NBINS + 1, np.int64)
    off[1:] = np.cumsum(nch)

    in_maps = [
        _pack_core_inputs(
            node_features[b], *per_core[b], nch, off,
            rel_weight, root_weight, rgcn_bias, lin_weight, lin_bias,
        )
        for b in range(B)
    ]
    return nch, in_maps


def kernel(node_features, edge_index, edge_type, rel_weight, root_weight,
           rgcn_bias, lin_weight, lin_bias, **_ignored):
    nch, in_maps = _prep(node_features, edge_index, edge_type, rel_weight,
                         root_weight, rgcn_bias, lin_weight, lin_bias)
    if nch not in _prog_cache:
        _prog_cache[nch] = build_program(nch)
    nc = _prog_cache[nch]
    res = run_bass_kernel_spmd(nc, in_maps, core_ids=list(range(B)))
    out = np.stack([res.results[b]["scores"].reshape(N) for b in range(B)])
    return out.astype(np.float32)


def kernel_profiled(node_features, edge_index, edge_type, rel_weight,
                    root_weight, rgcn_bias, lin_weight, lin_bias, **_ignored):
    """Run once with NTFF tracing; returns exec_time_ns (or None)."""
    import tempfile

    nch, in_maps = _prep(node_features, edge_index, edge_type, rel_weight,
                         root_weight, rgcn_bias, lin_weight, lin_bias)
    if nch not in _prog_cache:
        _prog_cache[nch] = build_program(nch)
    nc = _prog_cache[nch]
    tmpdir = tempfile.mkdtemp(prefix="rgcn_prof_")
    res = run_bass_kernel_spmd(
        nc, in_maps, core_ids=list(range(B)), trace=True, tmpdir=tmpdir
    )
    print(f"profile artifacts in {tmpdir}")
    return res.exec_time_ns


# revision 13
# speedup vs baseline: 3.0780x; 1.0366x over previous
"""RGCN graph-scoring kernel for Trainium2 (8 NeuronCores, one graph per core).

Math (per graph):
  out = relu(x @ root + bias + sum_r mean_r @ W_r);  scores = out @ lin + linb
  mean_r[n] = mean of x[src_e] over edges e with dst_e == n, type_e == r.

Device strategy per core (v2 — bf16 + 4 SWDGE queues + host-built one-hots):
  1. Phase 1: xw[src*8 + r_local] = (x @ W_r)[src] in bf16, staged to DRAM
     (two halves r<8 / r>=8 so gather indices fit in int16). PSUM->SBUF
     copies alternate between the Scalar and Vector engines.
  2. Phase 2, per dst tile t: dma_gather edge rows z_e = xw[src_e, type_e]
     (gathers round-robin across 4 SWDGE queues so descriptor generation
     runs on multiple Q7 core pairs concurrently), then accumulate
       acc[c', m] += z_chunk^T @ OH_chunk
     where OH_chunk[e, m] = alpha_e * (dstloc_e == m) is PRECOMPUTED ON THE
     HOST in bf16 and DMA-streamed (alpha_e = 1/cnt(type_e, dst_e) folds the
     mean normalization; padding slots have alpha 0 and index 0).
     acc is seeded by the root-weight matmul; relu+bias on ACT; scores via
     an M=1 matmul with the linear head.

Host side shards graphs across cores, sorts edges by (dst_tile, r_half) into
per-bin chunk counts shared across all 8 cores (max over cores, rounded up to
128) so the SPMD program is identical, and packs index/one-hot arrays.
"""

import sys

for _p in ("/opt/trn_rl_repo", "/root/.axon_site/_ro/trn_rl_repo"):
    if _p not in sys.path:
        sys.path.insert(0, _p)

import numpy as np
import ml_dtypes

import concourse.bacc as bacc
import concourse.mybir as mybir
from concourse.tile import TileContext
from concourse.bass_utils import run_bass_kernel_spmd

P = 128
B, N, C, R, E = 8, 4096, 128, 16, 65536
NT = N // P  # 32 dst node tiles
NH = 2  # r halves (int16 gather index limit: src*8+rl < 32768)
RH = R // NH  # 8 relations per half
NBINS = NT * NH
NQ = 4  # SWDGE queues

BF16 = ml_dtypes.bfloat16

_prog_cache = {}


def build_program(nch):
    """Build + compile the SPMD Bass program for per-bin chunk counts `nch`
    (tuple of NBINS ints, shared by all cores)."""
    nch = tuple(int(v) for v in nch)
    G = sum(nch)  # total 128-edge chunks
    off = np.zeros(NBINS + 1, np.int64)
    off[1:] = np.cumsum(nch)

    nc = bacc.Bacc("TRN2", num_swdge_queues=NQ)
    f32 = mybir.dt.float32
    bf16 = mybir.dt.bfloat16

    xT = nc.dram_tensor("xT", [P, N], bf16, kind="ExternalInput")
    wcat = nc.dram_tensor("wcat", [P, R * C], bf16, kind="ExternalInput")
    root = nc.dram_tensor("root", [P, C], bf16, kind="ExternalInput")
    bias = nc.dram_tensor("bias", [P, 1], f32, kind="ExternalInput")
    lin = nc.dram_tensor("lin", [P, 1], bf16, kind="ExternalInput")
    linb = nc.dram_tensor("linb", [1, 1], f32, kind="ExternalInput")
    gidx = nc.dram_tensor("gidx", [P, G * 8], mybir.dt.int16, kind="ExternalInput")
    oh = nc.dram_tensor("oh", [P, G * P], bf16, kind="ExternalInput")
    scores = nc.dram_tensor("scores", [1, N], f32, kind="ExternalOutput")

    with TileContext(nc) as tc:
        with (
            tc.tile_pool(name="const", bufs=1) as cpool,
            tc.tile_pool(name="stage", bufs=4) as spool,
            tc.tile_pool(name="z0", bufs=NT) as zpool0,
            tc.tile_pool(name="z1", bufs=6) as zpool1,
            tc.tile_pool(name="oh", bufs=3) as ohpool,
            tc.tile_pool(name="post", bufs=4) as ppool,
            tc.tile_pool(name="pxw", bufs=2, space="PSUM") as pxw_pool,
            tc.tile_pool(name="pacc", bufs=2, space="PSUM") as pacc_pool,
            tc.tile_pool(name="plin", bufs=2, space="PSUM") as plin_pool,
            tc.tile_pool(name="dram", bufs=1, space="DRAM") as dpool,
        ):
            # ---- resident inputs ----
            xT_t = cpool.tile([P, N], bf16)
            nc.sync.dma_start(out=xT_t[:], in_=xT[:])
            wcat_t = cpool.tile([P, R * C], bf16)
            nc.sync.dma_start(out=wcat_t[:], in_=wcat[:])
            root_t = cpool.tile([P, C], bf16)
            nc.sync.dma_start(out=root_t[:], in_=root[:])
            bias_t = cpool.tile([P, 1], f32)
            nc.sync.dma_start(out=bias_t[:], in_=bias[:])
            lin_t = cpool.tile([P, 1], bf16)
            nc.sync.dma_start(out=lin_t[:], in_=lin[:])
            linb_t = cpool.tile([1, 1], f32)
            nc.sync.dma_start(out=linb_t[:], in_=linb[:])
            idx_t = cpool.tile([P, G * 8], mybir.dt.int16)
            nc.sync.dma_start(out=idx_t[:], in_=gidx[:])

            # DRAM scratch: per-half transformed features, row = src*8 + r_local
            xw = [
                dpool.tile([N * RH, C], bf16, name=f"xw{h}", tag=f"xw{h}")
                for h in range(NH)
            ]

            # ---- phase 1: xw = x @ W_r (bf16), staged out to DRAM ----
            # h-major so all of xw[0] lands first and the h=0 gathers can
            # start while the h=1 half is still being computed. xw writes are
            # batched 4 src tiles per DMA to keep the Sync engine's issue rate
            # off the critical path.
            SG = 4  # src tiles per staged write
            for h in range(NH):
                for sg in range(NT // SG):
                    stg = spool.tile([P, SG, RH * C], bf16, tag="stage")
                    for si in range(SG):
                        st = sg * SG + si
                        pxw = pxw_pool.tile([P, RH * C], f32, space="PSUM", tag="pxw")
                        for g in range(2):
                            nc.tensor.matmul(
                                out=pxw[:, g * 512 : (g + 1) * 512],
                                lhsT=xT_t[:, st * P : (st + 1) * P],
                                rhs=wcat_t[:, h * RH * C + g * 512 : h * RH * C + (g + 1) * 512],
                                start=True,
                                stop=True,
                            )
                        # split the PSUM->SBUF cast across ACT and DVE so
                        # neither engine gates the PE
                        nc.scalar.activation(
                            out=stg[:, si, :512],
                            in_=pxw[:, :512],
                            func=mybir.ActivationFunctionType.Copy,
                        )
                        nc.vector.tensor_scalar(
                            out=stg[:, si, 512:],
                            in0=pxw[:, 512:],
                            scalar1=0.0,
                            scalar2=None,
                            op0=mybir.AluOpType.add,
                        )
                    dst_view = xw[h][:].rearrange(
                        "(ng s p rl) c -> ng p s rl c", ng=NT // SG, s=SG, p=P, rl=RH
                    )[sg]
                    nc.sync.dma_start(
                        out=dst_view,
                        in_=stg[:].rearrange("p s (rl c) -> p s rl c", rl=RH),
                    )

            # ---- phase 2: gather + aggregate per dst tile ----
            # All h=0 gathers are issued first: they only depend on xw[0], so
            # Q7 descriptor generation overlaps with phase 1's h=1 half.
            z_h0 = []
            for t in range(NT):
                b = t * NH
                cap = nch[b] * P
                z = zpool0.tile([P, nch[b], C], bf16, tag="z0")
                nc.gpsimd.dma_gather(
                    z[:],
                    xw[0][:],
                    idx_t[:, off[b] * 8 : off[b] * 8 + cap // 16],
                    cap,
                    cap,
                    C,
                    single_packet=False,
                    queue_num=t % NQ,
                )
                z_h0.append(z)

            scores_t = cpool.tile([1, N], f32)
            for t in range(NT):
                b0, b1 = t * NH, t * NH + 1
                tch = nch[b0] + nch[b1]  # chunks for this tile
                oh_t = ohpool.tile([P, tch * P], bf16, tag="oh")
                nc.scalar.dma_start(
                    out=oh_t[:], in_=oh[:, off[b0] * P : (off[b0] + tch) * P]
                )
                cap1 = nch[b1] * P
                z1 = zpool1.tile([P, nch[b1], C], bf16, tag="z1")
                nc.gpsimd.dma_gather(
                    z1[:],
                    xw[1][:],
                    idx_t[:, off[b1] * 8 : off[b1] * 8 + cap1 // 16],
                    cap1,
                    cap1,
                    C,
                    single_packet=False,
                    queue_num=t % NQ,
                )
                acc = pacc_pool.tile([P, P], f32, space="PSUM", tag="acc")
                # root term seeds the accumulator (start=True clears the bank)
                nc.tensor.matmul(
                    out=acc[:],
                    lhsT=root_t[:],
                    rhs=xT_t[:, t * P : (t + 1) * P],
                    start=True,
                    stop=False,
                )
                for h, zt in ((0, z_h0[t]), (1, z1)):
                    b = t * NH + h
                    for j in range(nch[b]):
                        g_loc = (off[b] - off[b0]) + j
                        nc.tensor.matmul(
                            out=acc[:],
                            lhsT=zt[:, j, :],
                            rhs=oh_t[:, g_loc * P : (g_loc + 1) * P],
                            start=False,
                            stop=(h == NH - 1 and j == nch[b] - 1),
                        )
                # relu(acc + bias) -> SBUF bf16
                relu_t = ppool.tile([P, P], bf16, tag="relu")
                nc.scalar.activation(
                    out=relu_t[:],
                    in_=acc[:],
                    func=mybir.ActivationFunctionType.Relu,
                    bias=bias_t[:, :1],
                )
                plin = plin_pool.tile([P, P], f32, space="PSUM", tag="plin")
                nc.tensor.matmul(
                    out=plin[:1, :],
                    lhsT=lin_t[:],
                    rhs=relu_t[:],
                    start=True,
                    stop=True,
                )
                nc.vector.tensor_scalar(
                    out=scores_t[:1, t * P : (t + 1) * P],
                    in0=plin[:1, :],
                    scalar1=linb_t[:1, :1],
                    scalar2=None,
                    op0=mybir.AluOpType.add,
                )
            nc.sync.dma_start(out=scores[:], in_=scores_t[:])

    nc.compile()
    return nc


def _bin_edges(ei, et):
    """Per-graph bin ids and per-edge fields (no padding decisions here)."""
    src = ei[0].astype(np.int64)
    dst = ei[1].astype(np.int64)
    et = et.astype(np.int64)
    cnt = np.bincount(et * N + dst, minlength=R * N).astype(np.float32)
    alpha_e = 1.0 / cnt[et * N + dst]
    t_e = dst >> 7
    h_e = et >> 3
    rl_e = et & 7
    binid = t_e * NH + h_e
    return src, dst, rl_e, alpha_e, binid


def _pack_core_inputs(x, src, dst, rl_e, alpha_e, binid, nch, off,
                      rel_w, root_w, rgcn_b, lin_w, lin_b):
    """Host-side prep for one graph given shared per-bin chunk counts."""
    G = int(off[-1])
    order = np.argsort(binid, kind="stable")
    counts = np.bincount(binid, minlength=NBINS)
    starts = np.zeros(NBINS, np.int64)
    starts[1:] = np.cumsum(counts)[:-1]
    # slot of each (sorted) edge inside the padded chunk layout
    pos = np.arange(E) - starts[binid[order]] + off[binid[order]] * P

    g = np.zeros(G * P, np.int16)  # pad slots gather row 0 (alpha 0)
    g[pos] = (src[order] * 8 + rl_e[order]).astype(np.int16)
    gidx = np.tile(g.reshape(-1, 16).T, (8, 1)).copy()

    A = np.zeros((G * P, P), np.float32)
    A[pos, (dst[order] & 127)] = alpha_e[order]
    oh = np.ascontiguousarray(
        A.reshape(G, P, P).transpose(1, 0, 2).reshape(P, G * P)
    ).astype(BF16)

    return {
        "xT": np.ascontiguousarray(x.T).astype(BF16),
        "wcat": np.ascontiguousarray(
            rel_w.transpose(1, 0, 2).reshape(C, R * C)
        ).astype(BF16),
        "root": np.ascontiguousarray(root_w).astype(BF16),
        "bias": np.ascontiguousarray(rgcn_b.reshape(C, 1)).astype(np.float32),
        "lin": np.ascontiguousarray(lin_w.reshape(C, 1)).astype(BF16),
        "linb": np.ascontiguousarray(lin_b.reshape(1, 1)).astype(np.float32),
        "gidx": gidx,
        "oh": oh,
    }


def _prep(node_features, edge_index, edge_type, rel_weight, root_weight,
          rgcn_bias, lin_weight, lin_bias):
    node_features = np.asarray(node_features, np.float32)
    edge_index = np.asarray(edge_index)
    edge_type = np.asarray(edge_type)
    rel_weight = np.asarray(rel_weight, np.float32)
    root_weight = np.asarray(root_weight, np.float32)
    rgcn_bias = np.asarray(rgcn_bias, np.float32)
    lin_weight = np.asarray(lin_weight, np.float32)
    lin_bias = np.asarray(lin_bias, np.float32)

    per_core = [
        _bin_edges(edge_index[b], edge_type[b]) for b in range(B)
    ]
    counts = np.stack(
        [np.bincount(pc[4], minlength=NBINS) for pc in per_core]
    )  # [B, NBINS]
    nch = tuple(int(v) for v in np.maximum(
        1, -(-counts.max(axis=0) // P)
    ))  # shared per-bin chunk counts
    off = np.zeros(NBINS + 1, np.int64)
    off[1:] = np.cumsum(nch)

    in_maps = [
        _pack_core_inputs(
            node_features[b], *per_core[b], nch, off,
            rel_weight, root_weight, rgcn_bias, lin_weight, lin_bias,
        )
        for b in range(B)
    ]
    return nch, in_maps


def kernel(node_features, edge_index, edge_type, rel_weight, root_weight,
           rgcn_bias, lin_weight, lin_bias, **_ignored):
    nch, in_maps = _prep(node_features, edge_index, edge_type, rel_weight,
                         root_weight, rgcn_bias, lin_weight, lin_bias)
    if nch not in _prog_cache:
        _prog_cache[nch] = build_program(nch)
    nc = _prog_cache[nch]
    res = run_bass_kernel_spmd(nc, in_maps, core_ids=list(range(B)))
    out = np.stack([res.results[b]["scores"].reshape(N) for b in range(B)])
    return out.astype(np.float32)


def kernel_profiled(node_features, edge_index, edge_type, rel_weight,
                    root_weight, rgcn_bias, lin_weight, lin_bias, **_ignored):
    """Run once with NTFF tracing; returns exec_time_ns (or None)."""
    import tempfile

    nch, in_maps = _prep(node_features, edge_index, edge_type, rel_weight,
                         root_weight, rgcn_bias, lin_weight, lin_bias)
    if nch not in _prog_cache:
        _prog_cache[nch] = build_program(nch)
    nc = _prog_cache[nch]
    tmpdir = tempfile.mkdtemp(prefix="rgcn_prof_")
    res = run_bass_kernel_spmd(
        nc, in_maps, core_ids=list(range(B)), trace=True, tmpdir=tmpdir
    )
    print(f"profile artifacts in {tmpdir}")
    return res.exec_time_ns
